# revision 6
# baseline (speedup 1.0000x reference)
"""ARX recurrence kernel for Trainium2 (8 NeuronCores, data-parallel).

Math: the reference runs out[:, t] = window @ w_ar + (u @ w_u + w_b) as a
sequential scan over 1008 steps.  Since the recurrence is linear, every
output timestep is a linear functional of X = [y | u | 1]:

    out[:, t] = X @ G[:, t]          G: [32, 1024]

G depends only on the 32-element weight vector, so it is computed on the
HOST in float64 (exact to fp32 working precision) and shipped to the
device as a 512 KB input, pre-replicated across the 4 partition
quadrants.  This removes the ~25 us serial on-device prologue that
previously delayed the first output DMA to t=33us.

The bulk work is a single [128, 32] x [32, 1024] matmul per 128-row
batch tile, executed as float32r (1 cyc/row on the PE instead of fp32's
4) with 4x row-tiling of the PE array (K=32 per quadrant), which makes
the kernel output-DMA-bound (32 MB/core at ~360 GB/s -> 93 us floor).

Batch <-> partition mapping: partition q of batch-tile s holds batch row
64*q + s (so the 512 KB y / 480 KB u inputs load as fully contiguous 4 KB
per-partition DMA chunks, and output rows are still contiguous 4 KB rows).
"""

import numpy as np

import concourse.bacc as bacc
import concourse.bass as bass
import concourse.mybir as mybir
import concourse.tile as tile
from concourse.masks import make_identity
from concourse.bass_utils import run_bass_kernel_spmd

N_CORES = 8
B_FULL = 65536
AR = 16          # ar order
NU = 15          # exogenous dim
K = 32           # regressor dim = AR + NU + 1
S = 1024         # sequence length
T_PRED = S - AR  # 1008 predicted steps

B = B_FULL // N_CORES      # 8192 rows per core
NTILES = B // 128          # 64 batch tiles of 128 rows
GROUPS = NTILES // 4       # 16 groups of 4 tiles (one 128x128 transpose each)
N_CHUNKS = 4               # input loaded in 4 chunks for pipelining
F32 = mybir.dt.float32
# dtype tag for the main matmuls: float32r = same fp32 bits, streamed at
# 1 cyc/row instead of 4 (TF32-like internal precision).  Measured rel
# err 1.1e-4 on HW vs the 2e-2 gate.
MM_DTYPE = mybir.dt.float32r


def _mm(ap):
    """View an AP in the main-matmul dtype (no-op for plain fp32)."""
    return ap if MM_DTYPE == F32 else ap.bitcast(MM_DTYPE)


# scheduling knobs (module-level so experiments can tweak them).
X4_BUFS = 6
XT_BUFS = 8
OUT_BUFS = 4
PS_BUFS = 6
PSXT_BUFS = 2
ASM_ENGINE = "gpsimd"   # which engine assembles X4 ([y|u|1] copies)
DO_MM = True            # False: skip main matmuls (timing ablation only)
IN_ENGINE = "gpsimd"    # engine issuing input loads (SWDGE keeps HWDGE free)
OUT_DUAL = True         # two 1MB stores per group, one on each HWDGE ring
COPY_53 = True          # split psum drains 5 DVE / 3 ACT


def host_g(w):
    """Compute G [32, S] on the host in float64.

    out[:, 0:AR] = y;  out[:, AR+t] = y @ a_t + (u @ w_u + w_b) * b_t.
    Row layout matches X = [y | u | 1]:
      G[0:16, :]  = y coefficients (identity block for the prefix)
      G[16:31, t] = w_u * b_t
      G[31, t]    = w_b * b_t
    """
    w = np.asarray(w, np.float64)
    w_ar, w_u, w_b = w[:AR], w[AR : AR + NU], w[AR + NU]
    # Wc [AR, AR+1] maps [y, const] -> current window; e_const adds const.
    Wc = np.zeros((AR, AR + 1))
    Wc[:, :AR] = np.eye(AR)
    preds = np.empty((T_PRED, AR + 1))
    for t in range(T_PRED):
        pc = w_ar @ Wc
        pc[AR] += 1.0
        preds[t] = pc
        Wc = np.concatenate([Wc[1:], pc[None, :]], axis=0)
    G = np.zeros((K, S), np.float64)
    G[:AR, :AR] = np.eye(AR)
    G[:AR, AR:] = preds[:, :AR].T
    G[AR : AR + NU, AR:] = np.outer(w_u, preds[:, AR])
    G[K - 1, AR:] = w_b * preds[:, AR]
    return G.astype(np.float32)


def build_nc(b=B, reps=1):
    """Build the per-core Bass program (SPMD: same program, 8 shards).

    reps>1 unrolls the whole main loop multiple times inside one NEFF
    (writes the same outputs each rep) — used only for steady-state HW
    timing, never for grading."""
    ntiles = b // 128
    groups = ntiles // 4
    n_chunks = max(1, min(N_CHUNKS, groups))
    grp_per_chunk = groups // n_chunks
    s_per_part = b // 128  # rows per partition in the pack layout

    nc = bacc.Bacc("TRN2", target_bir_lowering=False, debug=False)

    y_d = nc.dram_tensor("y", [b, AR], F32, kind="ExternalInput").ap()
    u_d = nc.dram_tensor("u", [b, NU], F32, kind="ExternalInput").ap()
    g_d = nc.dram_tensor("g", [128, S], F32, kind="ExternalInput").ap()
    out_d = nc.dram_tensor("out", [b, S], F32, kind="ExternalOutput").ap()

    # pack views: partition q <-> batch rows [q*s_per_part, (q+1)*s_per_part)
    y_pack = y_d.rearrange("(q s) k -> q (s k)", q=128)    # [128, s_per_part*16]
    u_pack = u_d.rearrange("(q s) k -> q (s k)", q=128)    # [128, s_per_part*15]
    out_flat = out_d.rearrange("(q s) t -> q (s t)", q=128)  # [128, s_per_part*1024]

    from contextlib import ExitStack
    with tile.TileContext(nc) as tc, ExitStack() as ctx:
        singles = ctx.enter_context(tc.tile_pool(name="singles", bufs=1))
        x4_pool = ctx.enter_context(tc.tile_pool(name="x4", bufs=X4_BUFS))
        xt_pool = ctx.enter_context(tc.tile_pool(name="xt", bufs=XT_BUFS))
        out_pool = ctx.enter_context(tc.tile_pool(name="outsb", bufs=OUT_BUFS))
        ps_pool = ctx.enter_context(
            tc.tile_pool(name="ps", bufs=PS_BUFS, space="PSUM"))
        psxt_pool = ctx.enter_context(
            tc.tile_pool(name="psxt", bufs=PSXT_BUFS, space="PSUM"))

        in_eng = getattr(nc, IN_ENGINE)

        # G, host-computed, pre-replicated across the 4 partition quadrants.
        # Split into halves so the h=0 matmuls can start before the h=1
        # columns land.
        G_rep = singles.tile([128, S], F32, tag="Grep")
        in_eng.dma_start(out=_mm(G_rep[:, 0:512]), in_=_mm(g_d[:, 0:512]))
        in_eng.dma_start(out=_mm(G_rep[:, 512:S]), in_=_mm(g_d[:, 512:S]))

        # identity for PE transposes
        ident = singles.tile([128, 128], F32, tag="ident")
        make_identity(nc, ident[:, :])

        # --- input loads (chunked for pipelining) -----------------------
        ychunks, uchunks = [], []
        ccols_y = grp_per_chunk * 4 * AR   # cols of y_pack per chunk
        ccols_u = grp_per_chunk * 4 * NU
        for c in range(n_chunks):
            yc = singles.tile([128, ccols_y], F32, tag=f"ypack{c}")
            in_eng.dma_start(
                out=yc[:, :], in_=y_pack[:, c * ccols_y : (c + 1) * ccols_y])
            ychunks.append(yc)
            uc = singles.tile([128, ccols_u], F32, tag=f"upack{c}")
            in_eng.dma_start(
                out=uc[:, :], in_=u_pack[:, c * ccols_u : (c + 1) * ccols_u])
            uchunks.append(uc)

        # --- main loop: one group = 4 batch tiles = one 128x128 transpose
        for g in [g for _ in range(reps) for g in range(groups)]:
            c, gl = divmod(g, grp_per_chunk)

            # assemble X4 [128, 4, 32] = [y | u | 1] for 4 tiles
            X4 = x4_pool.tile([128, 128], F32, tag="x4")
            x4v = X4[:, :].rearrange("p (a k) -> p a k", a=4)
            yv = ychunks[c][:, gl * 4 * AR : (gl + 1) * 4 * AR].rearrange(
                "p (a k) -> p a k", a=4)
            uv = uchunks[c][:, gl * 4 * NU : (gl + 1) * 4 * NU].rearrange(
                "p (a k) -> p a k", a=4)
            asm = getattr(nc, ASM_ENGINE)
            asm.tensor_copy(out=x4v[:, :, 0:AR], in_=yv)
            asm.tensor_copy(out=x4v[:, :, AR : AR + NU], in_=uv)
            asm.memset(x4v[:, :, K - 1 : K], 1.0)

            out_sb = out_pool.tile([128, 4 * S], F32, tag="outsb")

            # transpose -> XT4 [128,128]: rows 32j..32j+31 = X_j^T
            ps_xt = psxt_pool.tile([128, 128], F32, tag="psxt")
            nc.tensor.transpose(ps_xt[:, :], X4[:, :], ident[:, :])
            XT4 = xt_pool.tile([128, 128], F32, tag="xt")
            nc.vector.tensor_copy(out=_mm(XT4[:, :]), in_=ps_xt[:, :])

            # 8 row-tiled matmuls (4 quadrants x 2 column halves)
            for j in range(4):
                for h in range(2):
                    ps = ps_pool.tile([128, 512], F32, tag="ps")
                    if DO_MM:
                        nc.tensor.matmul(
                            ps[:, :],
                            _mm(XT4[32 * j : 32 * (j + 1), :]),
                            _mm(G_rep[32 * j : 32 * (j + 1),
                                      512 * h : 512 * (h + 1)]),
                            start=True, stop=True,
                            tile_position=(32 * j, 0),
                        )
                    else:
                        nc.vector.memset(ps[:, :], 0.0)
                    dst = out_sb[:, j * S + 512 * h : j * S + 512 * (h + 1)]
                    idx = j * 2 + h
                    on_dve = (idx < 5) if COPY_53 else ((j + h) % 2 == 0)
                    if on_dve:
                        nc.vector.tensor_copy(out=dst, in_=ps[:, :])
                    else:
                        nc.scalar.copy(out=dst, in_=ps[:, :])

            # output stores address DRAM through the flat per-partition view
            # so each partition's 2 adjacent rows form ONE 8KB descriptor
            # (128 descriptors per store instead of 256)
            g0 = 4 * g * S
            if OUT_DUAL:
                # two 1MB stores per group, one on each HWDGE ring, so both
                # rings stay busy every group
                nc.sync.dma_start(
                    out=out_flat[:, g0 : g0 + 2 * S], in_=out_sb[:, 0 : 2 * S])
                nc.scalar.dma_start(
                    out=out_flat[:, g0 + 2 * S : g0 + 4 * S],
                    in_=out_sb[:, 2 * S : 4 * S])
            else:
                out_eng = nc.sync if g % 2 == 0 else nc.scalar
                out_eng.dma_start(
                    out=out_flat[:, g0 : g0 + 4 * S], in_=out_sb[:, :])

    nc.compile()
    return nc


_NC_CACHE = {}


def _get_nc(b):
    if b not in _NC_CACHE:
        _NC_CACHE[b] = build_nc(b)
    return _NC_CACHE[b]


def make_in_maps(y, u, w):
    """Per-core input dicts for run_bass_kernel_spmd / the slope bench."""
    y = np.ascontiguousarray(np.asarray(y), dtype=np.float32)
    u = np.ascontiguousarray(np.asarray(u), dtype=np.float32)
    w = np.ascontiguousarray(np.asarray(w), dtype=np.float32)
    g32 = host_g(w)                       # [32, S] f32
    g_rep = np.ascontiguousarray(np.tile(g32, (4, 1)))  # [128, S]
    return [
        {"y": y[i * B : (i + 1) * B], "u": u[i * B : (i + 1) * B],
         "g": g_rep}
        for i in range(N_CORES)
    ]


def kernel(y, u, w):
    assert np.asarray(y).shape == (B_FULL, AR)
    assert np.asarray(u).shape == (B_FULL, NU)
    nc = _get_nc(B)
    in_maps = make_in_maps(y, u, w)
    res = run_bass_kernel_spmd(nc, in_maps, list(range(N_CORES)))
    return np.concatenate(
        [res.results[i]["out"] for i in range(N_CORES)], axis=0)
